# revision 6
# baseline (speedup 1.0000x reference)
"""Trainium2 Bass kernel for CuGraphRelGraphConv (basis-decomposed
relational graph conv) on 8 NeuronCores — dense-gather design.

Math: msg_e = coeff[etype_e] (x) feat[src_e];  agg = segsum(msg, dst);
      h = agg.reshape(N,128) @ W.reshape(128,64) + bias + feat @ loop_w.

Host schedule (all metadata precomputed from src/dst/etypes):
  - dst nodes dealt by degree to 8 cores x 392 windows of 32 slots; within
    each deal round the 8 same-rank nodes are permuted across cores to
    balance per-(window, src-bank) edge counts -> one SPMD program padded
    only ~3% over the true edge count.
  - per core: 14 groups x 28 windows; edge slot stream ordered
    (group, src-bank, window, src), dense; per-(g,b) segment sizes padded
    to the cross-core max (128-aligned, pad rows gather idx 0).
  - gather: per-(g,b) segments split into 1920-idx sub-calls (121 descs,
    under the 128-deep SWDGE ring) round-robin over all 4 SWDGE queues;
    queue-balanced and continuously fed (~2.3 ns/row sustained).
  - scatter: per "appearance" (128-edge tile x window) one 64-col matmul
    gt_chunk^T @ S01 accumulating into an 8-window PSUM pack [64, 512];
    S01 = onehot(dstl)*coeff built on DVE, sentinel dstl=32 masks foreign
    slots of tiles that straddle window boundaries.
  - h[slot, o] = sum_b agg_b^T @ W_b + [featT|1] @ [loop_w; bias] per
    window; packs of 8 windows copied out via the scalar HWDGE queue.
"""
import sys

sys.path.insert(0, "/opt/trn_rl_repo")

import numpy as np
import ml_dtypes

import concourse.bass as bass
import concourse.bacc as bacc
import concourse.mybir as mybir
from concourse.bass_utils import run_bass_kernel_spmd
from concourse.tile import TileContext

import numpy as np
N_NODES = 100000
N_EDGES = 1600000
K = 8
WIN = 32
NG = 14           # groups per core
_wins_per_core = -(-(-(-N_NODES // WIN) // K))
GW = -(-_wins_per_core // NG)   # windows per group
NW = NG * GW      # windows per core
NB = 4            # src banks (int16 gather reach)
BANK = 32768
NPC = NW * WIN
SUBCALL = 1920   # 121 descriptors < 128-deep SWDGE ring
SENT = float(WIN)  # dstl sentinel


def assign_nodes(dst, src):
    """Deal nodes by degree to (core, window, slot); within each round's
    window position, permute the 8 candidate nodes across cores to balance
    cumulative per-(window, src-bank) edge counts (cuts cross-core pad)."""
    deg = np.bincount(dst, minlength=N_NODES)
    degb = np.zeros((N_NODES, NB), np.int64)
    np.add.at(degb, (dst, src >> 15), 1)
    order = np.argsort(-deg, kind="stable")
    nwt = K * NW
    winf = np.empty(N_NODES, np.int64)
    slot = np.empty(N_NODES, np.int64)
    cum = np.zeros((K, NW, NB), np.float64)
    r = 0
    for off in range(0, N_NODES, nwt):
        ch = order[off: off + nwt]
        # nodes_rw[w, k] = node that plain snake would put at col k*NW+w
        nodes_rw = np.full((NW, K), -1, np.int64)
        cols = np.arange(len(ch))
        if r % 2 == 1:
            cols = nwt - 1 - cols
        nodes_rw[cols % NW, cols // NW] = ch
        # greedy per window position: biggest node first, to the core
        # minimizing sum_b (cum + deg)^2
        ordk = np.argsort(-deg[np.maximum(nodes_rw, 0)]
                          - 10**9 * (nodes_rw < 0), axis=1)   # [NW, K]
        used = np.zeros((NW, K), bool)
        rows = np.arange(NW)
        for step in range(K):
            v = nodes_rw[rows, ordk[:, step]]                 # [NW]
            ok = v >= 0
            d = degb[np.maximum(v, 0)]                        # [NW, NB]
            cost = ((cum.transpose(1, 0, 2) + d[:, None, :]) ** 2
                    ).sum(axis=2)                             # [NW, K]
            cost[used] = np.inf
            kk = np.argmin(cost, axis=1)                      # [NW]
            used[rows, kk] = True
            vv, kks, ws = v[ok], kk[ok], rows[ok]
            winf[vv] = kks * NW + ws
            slot[vv] = r
            cum[kks, ws] += degb[vv]
        r += 1
    assert slot.max() < WIN
    return winf, slot


def make_schedule(src, dst, etypes, coeff):
    src = np.asarray(src, np.int64)
    dst = np.asarray(dst, np.int64)
    etypes = np.asarray(etypes, np.int64)
    coeff = np.asarray(coeff, np.float32)

    winf, slot = assign_nodes(dst, src)
    core_of, w_of = winf // NW, winf % NW

    ek = core_of[dst]                  # core
    ew = w_of[dst]                     # window in core
    eg = ew // GW                      # group
    ewl = ew % GW                      # window in group
    eb = src >> 15                     # bank
    eidx = (src & (BANK - 1)).astype(np.int64)
    edstl = slot[dst]                  # 0..63
    ecc = coeff[etypes]                # [E, 2]

    # counts per (k, g, b, wl)
    key = ((ek * NG + eg) * NB + eb) * GW + ewl
    C = np.bincount(key, minlength=K * NG * NB * GW) \
        .reshape(K, NG, NB, GW).astype(np.int64)
    S = C.sum(axis=3)                                  # [K, NG, NB]
    Sstar = 128 * np.ceil(S.max(axis=0) / 128).astype(np.int64)  # [NG, NB]

    starts = np.cumsum(C, axis=3) - C                  # excl cumsum [K,NG,NB,GW]
    ends = starts + C
    lo = starts.min(axis=0)                            # [NG, NB, GW]
    hi = ends.max(axis=0)

    seg_off = np.cumsum(Sstar, axis=1) - Sstar         # [NG, NB] within group
    GS = Sstar.sum(axis=1)                             # group sizes
    grp_off = np.cumsum(GS) - GS
    S_total = int(GS.sum())

    # --- edge slot positions ---
    # order edges by (k, g, b, wl, src); per-bucket running position
    perm = np.lexsort((src, ewl, eb, eg, ek))
    k_s, g_s, b_s, wl_s = ek[perm], eg[perm], eb[perm], ewl[perm]
    bucket = ((k_s * NG + g_s) * NB + b_s) * GW + wl_s
    change = np.r_[True, bucket[1:] != bucket[:-1]]
    run_start = np.flatnonzero(change)
    run_id = np.cumsum(change) - 1
    pos = np.arange(len(perm)) - run_start[run_id]
    spos = (grp_off[g_s] + seg_off[g_s, b_s]
            + starts[k_s, g_s, b_s, wl_s] + pos)       # slot within core

    # per-core slot arrays
    idx_stream = np.zeros((K, S_total), np.int16)
    wl_slot = np.full((K, S_total), -1, np.int64)
    dstl_slot = np.full((K, S_total), WIN, np.float32)
    cc_slot = np.zeros((K, S_total, 2), np.float32)
    idx_stream[k_s, spos] = eidx[perm].astype(np.int16)
    wl_slot[k_s, spos] = wl_s
    dstl_slot[k_s, spos] = edstl[perm]
    cc_slot[k_s, spos] = ecc[perm]

    # --- appearances ---
    # per (g, w): ordered list over b, tiles t in seg (g,b) where
    # [lo, hi) of (g,b,w) intersects tile t. Order: (g, w, b, t).
    apps = []            # dicts: g, wl, b, t, chunk (within group), a (index)
    app_ranges = {}      # (g, wl) -> (a0, a1) contiguous? order by (g,w)
    for g in range(NG):
        for wl in range(GW):
            first = len(apps)
            for b in range(NB):
                l, h = lo[g, b, wl], hi[g, b, wl]
                if h <= l:
                    continue
                t0, t1 = l // 128, (h - 1) // 128 + 1
                for t in range(t0, t1):
                    apps.append(dict(
                        g=g, wl=wl, b=b, t=t,
                        chunk=(seg_off[g, b] // 128) + t))
            app_ranges[(g, wl)] = (first, len(apps))
    A = len(apps)

    # appearance metadata arrays [K, 128, A]
    dstl_app = np.full((K, 128, A), SENT, np.float32)
    cc_app = np.zeros((K, 128, A, 2), np.float32)
    for a, ap in enumerate(apps):
        g, wl, b, t = ap["g"], ap["wl"], ap["b"], ap["t"]
        s0 = grp_off[g] + seg_off[g, b] + 128 * t
        sl = slice(s0, s0 + 128)
        m = wl_slot[:, sl] == wl                       # [K, 128]
        dstl_app[:, :, a] = np.where(m, dstl_slot[:, sl], SENT)
        cc_app[:, :, a, :] = cc_slot[:, sl, :]

    # --- gather calls ---
    calls = []   # (g, col0, ncols16, chunk0, nchunks, queue)
    q = 0
    for g in range(NG):
        for b in range(NB):
            sz = int(Sstar[g, b])
            off = 0
            while off < sz:
                nn = min(SUBCALL, sz - off)
                s0 = grp_off[g] + seg_off[g, b] + off
                calls.append(dict(
                    g=g, idx0=int(s0), nidx=int(nn),
                    chunk0=int((seg_off[g, b] + off) // 128),
                    queue=q % 4))
                q += 1
                off += nn

    return dict(
        winf=winf, slot=slot, C=C, Sstar=Sstar, seg_off=seg_off, GS=GS,
        grp_off=grp_off, S_total=S_total, idx_stream=idx_stream,
        apps=apps, app_ranges=app_ranges, A=A,
        dstl_app=dstl_app, cc_app=cc_app, calls=calls,
        wl_slot=wl_slot,
    )



BF16 = ml_dtypes.bfloat16
PACK = 512 // (2 * WIN)       # windows per PSUM pack ([64, 512] fp32 bank)


def build_program(sched):
    import os
    stage = int(os.environ.get("K2_STAGE", "9"))
    dt = mybir.dt
    GS = sched["GS"]
    grp_off = sched["grp_off"]
    S_total = sched["S_total"]
    A = sched["A"]
    apps = sched["apps"]
    app_ranges = sched["app_ranges"]
    calls = sched["calls"]
    seg_off = sched["seg_off"]

    calls_by_g = [[c for c in calls if c["g"] == g] for g in range(NG)]

    nc = bacc.Bacc("TRN2", target_bir_lowering=False, debug=False,
                   num_devices=K, num_swdge_queues=4)

    table_d = nc.dram_tensor("table", [N_NODES, 128], dt.bfloat16,
                             kind="ExternalInput").ap()
    idx_d = nc.dram_tensor("idx", [128, S_total // 16], dt.int16,
                           kind="ExternalInput").ap()
    dstl_d = nc.dram_tensor("dstl", [128, A], dt.bfloat16,
                            kind="ExternalInput").ap()
    cc_d = nc.dram_tensor("cc", [128, 2 * A], dt.bfloat16,
                          kind="ExternalInput").ap()
    featT_d = nc.dram_tensor("featT", [65, NPC], dt.bfloat16,
                             kind="ExternalInput").ap()
    wmat_d = nc.dram_tensor("wmat", [64, 2 * 64], dt.bfloat16,
                            kind="ExternalInput").ap()   # [d, (b,o)]
    lw_d = nc.dram_tensor("lw65", [65, 64], dt.bfloat16,
                          kind="ExternalInput").ap()
    iota_d = nc.dram_tensor("iota", [128, WIN], dt.bfloat16,
                            kind="ExternalInput").ap()
    out_d = nc.dram_tensor("out", [NPC, 64], dt.float32,
                           kind="ExternalOutput").ap()

    max_gchunks = int(max(GS)) // 128
    # max appearances in any pack (for tile sizing)
    pack_na = []
    for g in range(NG):
        for p in range(0, GW, PACK):
            ws = range(p, min(p + PACK, GW))
            a0 = app_ranges[(g, ws[0])][0]
            a1 = app_ranges[(g, ws[-1])][1]
            pack_na.append(a1 - a0)
    max_na = max(pack_na)

    with TileContext(nc) as tc:
        with (
            tc.tile_pool(name="const", bufs=1) as cpool,
            tc.tile_pool(name="gidx", bufs=2) as ipool,
            tc.tile_pool(name="gather", bufs=2) as gpool,
            tc.tile_pool(name="sel", bufs=2) as spool,
            tc.tile_pool(name="aggsb", bufs=2) as apool,
            tc.tile_pool(name="hout", bufs=2) as hpool,
            tc.tile_pool(name="psum_a", bufs=1, space="PSUM") as pa,
            tc.tile_pool(name="psum_h", bufs=2, space="PSUM") as ph,
        ):
            dstl_t = cpool.tile([128, A], dt.bfloat16)
            nc.scalar.dma_start(out=dstl_t[:], in_=dstl_d[:])
            cc_t = cpool.tile([128, 2 * A], dt.bfloat16)
            nc.scalar.dma_start(out=cc_t[:], in_=cc_d[:])
            featT_t = cpool.tile([65, NPC], dt.bfloat16)
            nc.scalar.dma_start(out=featT_t[:], in_=featT_d[:])
            wmat_t = cpool.tile([64, 2 * 64], dt.bfloat16)
            nc.scalar.dma_start(out=wmat_t[:], in_=wmat_d[:])
            lw_t = cpool.tile([65, 64], dt.bfloat16)
            nc.scalar.dma_start(out=lw_t[:], in_=lw_d[:])
            iota_t = cpool.tile([128, WIN], dt.bfloat16)
            nc.scalar.dma_start(out=iota_t[:], in_=iota_d[:])

            for g in range(NG):
                gsz = int(GS[g])
                git = ipool.tile([128, max_gchunks * 8], dt.int16, tag="i")
                nc.sync.dma_start(
                    out=git[:, : gsz // 16],
                    in_=idx_d[:, int(grp_off[g]) // 16:
                              (int(grp_off[g]) + gsz) // 16])
                gt = gpool.tile([128, max_gchunks, 128], dt.bfloat16, tag="g")
                for c in calls_by_g[g]:
                    i0 = (c["idx0"] - int(grp_off[g])) // 16
                    nch = c["nidx"] // 128
                    nc.gpsimd.dma_gather(
                        out_ap=gt[:, c["chunk0"]: c["chunk0"] + nch, :],
                        in_ap=table_d[c["bank"] * BANK:
                                      min((c["bank"] + 1) * BANK, N_NODES), :],
                        idxs_ap=git[:, i0: i0 + c["nidx"] // 16],
                        num_idxs=c["nidx"],
                        num_idxs_reg=c["nidx"],
                        elem_size=128,
                        queue_num=c["queue"],
                        single_packet=False,
                    )

                for p0 in range(0, GW, PACK):
                    if stage < 2:
                        break
                    ws = list(range(p0, min(p0 + PACK, GW)))
                    a0 = app_ranges[(g, ws[0])][0]
                    a1 = app_ranges[(g, ws[-1])][1]
                    na = a1 - a0
                    if na > 0:
                        oh = spool.tile([128, max_na, WIN], dt.bfloat16,
                                        tag="oh")
                        s01 = spool.tile([128, max_na, 2, WIN], dt.bfloat16,
                                         tag="s01")
                        nc.vector.tensor_tensor(
                            out=oh[:, :na, :],
                            in0=dstl_t[:, a0:a1].unsqueeze(-1)
                                .to_broadcast([128, na, WIN]),
                            in1=iota_t[:].unsqueeze(1)
                                .to_broadcast([128, na, WIN]),
                            op=mybir.AluOpType.is_equal,
                        )
                        nc.vector.tensor_tensor(
                            out=s01[:, :na, :, :],
                            in0=oh[:, :na, :].unsqueeze(2)
                                .to_broadcast([128, na, 2, WIN]),
                            in1=cc_t[:, 2 * a0: 2 * a1]
                                .rearrange("p (a c) -> p a c", c=2)
                                .unsqueeze(-1).to_broadcast([128, na, 2, WIN]),
                            op=mybir.AluOpType.mult,
                        )
                    aps = pa.tile([64, PACK * 2 * WIN], dt.float32,
                                  tag=f"a{p0 // PACK}", name=f"aps{p0}")
                    for wl in ws:
                        wa0, wa1 = app_ranges[(g, wl)]
                        col = (wl - p0) * 2 * WIN
                        for a in range(wa0, wa1):
                            ap_ = apps[a]
                            nc.tensor.matmul(
                                out=aps[:, col: col + 2 * WIN],
                                lhsT=gt[:, ap_["chunk"], 0:64],
                                rhs=s01[:, a - a0, :, :]
                                    .rearrange("p c w -> p (c w)"),
                                start=(a == wa0),
                                stop=(a == wa1 - 1),
                            )
                    aggs = apool.tile([64, PACK * 2 * WIN], dt.bfloat16,
                                      tag="aggs")
                    ncols = len(ws) * 2 * WIN
                    nc.scalar.activation(
                        out=aggs[:, :ncols], in_=aps[:, :ncols],
                        func=mybir.ActivationFunctionType.Copy)
                    hps = ph.tile([WIN, PACK, 64], dt.float32, tag="h")
                    for wl in ws:
                        wa0, wa1 = app_ranges[(g, wl)]
                        has = wa1 > wa0
                        col = (wl - p0) * 2 * WIN
                        hslice = hps[:, wl - p0, :]
                        if has:
                            for b2 in range(2):
                                nc.tensor.matmul(
                                    out=hslice,
                                    lhsT=aggs[:, col + b2 * WIN:
                                              col + (b2 + 1) * WIN],
                                    rhs=wmat_t[:, b2 * 64: b2 * 64 + 64],
                                    start=(b2 == 0),
                                    stop=False,
                                )
                        w_glob = g * GW + wl
                        nc.tensor.matmul(
                            out=hslice,
                            lhsT=featT_t[:, w_glob * WIN:
                                         w_glob * WIN + WIN],
                            rhs=lw_t[:],
                            start=(not has),
                            stop=True,
                        )
                    hs = hpool.tile([WIN, PACK, 64], dt.float32, tag="hs")
                    nc.scalar.activation(
                        out=hs[:, : len(ws), :], in_=hps[:, : len(ws), :],
                        func=mybir.ActivationFunctionType.Copy)
                    r0 = (g * GW + p0) * WIN
                    nc.scalar.dma_start(
                        out=out_d[r0: r0 + len(ws) * WIN, :]
                            .rearrange("(w s) o -> s w o", w=len(ws)),
                        in_=hs[:, : len(ws), :])

    nc.compile()
    return nc


def make_inputs(sched, feat, W, coeff, h_bias, loop_weight):
    winf, slot = sched["winf"], sched["slot"]
    core_of, w_of = winf // NW, winf % NW

    table = np.zeros((N_NODES, 128), BF16)
    table[:, 0:64] = feat.astype(BF16)

    wmat = np.ascontiguousarray(
        W.transpose(1, 0, 2).reshape(64, 2 * 64)).astype(BF16)
    lw65 = np.concatenate(
        [loop_weight.astype(np.float32), h_bias[None].astype(np.float32)],
        0).astype(BF16)
    iota = np.tile(np.arange(WIN, dtype=np.float32)[None],
                   (128, 1)).astype(BF16)

    # idx wrap: [S_total] -> [128, S/16]
    idx = sched["idx_stream"]                     # [K, S_total] int16
    S = idx.shape[1]
    idxw = np.ascontiguousarray(
        idx.reshape(K, S // 16, 16).transpose(0, 2, 1))       # [K, 16, S/16]
    idxw = np.tile(idxw, (1, 8, 1))                           # [K, 128, S/16]

    dstl_app = sched["dstl_app"].astype(BF16)                 # [K, 128, A]
    cc_app = np.ascontiguousarray(
        sched["cc_app"].reshape(K, 128, -1)).astype(BF16)     # [K, 128, 2A]

    in_maps = []
    for k in range(K):
        fT = np.zeros((65, NPC), np.float32)
        fT[64, :] = 1.0
        mine = core_of == k
        rows = w_of[mine] * WIN + slot[mine]
        fT[0:64, rows] = feat[mine].T
        in_maps.append({
            "table": table,
            "idx": idxw[k],
            "dstl": dstl_app[k],
            "cc": cc_app[k],
            "featT": fT.astype(BF16),
            "wmat": wmat,
            "lw65": lw65,
            "iota": iota,
        })
    return in_maps


def run(feat, W, coeff, h_bias, loop_weight, src, dst, etypes,
        trace=False):
    sched = make_schedule(np.asarray(src, np.int64),
                          np.asarray(dst, np.int64),
                          np.asarray(etypes, np.int64),
                          np.asarray(coeff, np.float32))
    # annotate calls with bank (needed for in_ap slicing)
    for c in sched["calls"]:
        g = c["g"]
        # recover bank from chunk0 via seg_off
        so = sched["seg_off"][g] // 128
        b = int(np.searchsorted(so, c["chunk0"], side="right") - 1)
        c["bank"] = b
    nc = build_program(sched)
    in_maps = make_inputs(sched, np.asarray(feat, np.float32),
                          np.asarray(W, np.float32),
                          np.asarray(coeff, np.float32),
                          np.asarray(h_bias, np.float32),
                          np.asarray(loop_weight, np.float32))
    res = run_bass_kernel_spmd(nc, in_maps, list(range(K)), trace=trace)
    outs = np.stack([res.results[k]["out"] for k in range(K)])  # [K, NPC, 64]
    winf, slot = sched["winf"], sched["slot"]
    core_of, w_of = winf // NW, winf % NW
    h = outs[core_of, w_of * WIN + slot, :]
    return h.astype(np.float32), res


def kernel(feat, W, coeff, h_bias, loop_weight, src, dst, etypes):
    h, _ = run(feat, W, coeff, h_bias, loop_weight, src, dst, etypes)
    return h
